# revision 20
# baseline (speedup 1.0000x reference)
"""Trainium2 Bass kernel for CrossTrans block (dense_transformer).

Computation (per batch b):
  x   = xx[:, 288:384]                      # query stream  [96, N]
  q   = Wq'@x + qb ; k = Wk'@xx + kb ; v = Wv'@xx + vb
  attn= softmax(q_h^T k_h) per head ; av = v_h @ attn^T
  y   = relu(Wo'@(x + Wp'@relu(av_norm)) + ob')
BN scales folded into weights on host; p_bias folded into o_bias.

Sharding: 8 cores = 4 batches x 2 query-halves; k/v recomputed per half.

Device layout: scores transposed [keys->partitions, queries->free] so AV
consumes exp(scores) without transposing the attention matrix. Softmax
denominators via ones-matmul, col-tiled, row-replicated x32 so they align
partition-wise with AV output for the normalize op. exp skips max
subtraction (|logit| <~ 70 fits fp32/bf16 range).

Precision: scores use a 3-term split-bf16 matmul
  [k_hi; k_lo; k_hi]^T . [q_hi; q_hi; q_lo]  (K=48, fp32-grade logits)
with the terms stacked at two 64-row strips for tile_position concurrency.
e and v are bf16; k/q convs and the output stage are fp32.
"""

import numpy as np

NUM_HEADS = 6
KD = 16
DH = 32
B, C, Himg, Wimg = 4, 384, 48, 48
N = Himg * Wimg          # 2304
NH = N // 2              # 1152 queries per core
DIM_S = C // 4           # 96
NHKD = NUM_HEADS * KD    # 96
DHALL = NUM_HEADS * DH   # 192
NCORES = 8

NCH = 384                # query chunk (free dim of score matmuls)
NNC = NH // NCH          # 3 query chunks per core
MCH = 128                # key chunk (partition tile)
NMCH = N // MCH          # 18 key chunks
MG = 3                   # key chunks per exp group (3 psum banks)
NG = NMCH // MG          # 6 groups
KT = C // 128            # 3 contraction tiles over channels


def build_nc():
    import concourse.bacc as bacc
    import concourse.mybir as mybir
    from concourse.tile import TileContext

    fp32 = mybir.dt.float32
    bf16 = mybir.dt.bfloat16
    AF = mybir.ActivationFunctionType
    OP = mybir.AluOpType

    nc = bacc.Bacc("TRN2", target_bir_lowering=False)

    xx_d = nc.dram_tensor("xx", [C, N], fp32, kind="ExternalInput")
    xh_d = nc.dram_tensor("xh", [DIM_S, NH], fp32, kind="ExternalInput")
    wk_d = nc.dram_tensor("wkT", [C, NHKD], fp32, kind="ExternalInput")
    wv_d = nc.dram_tensor("wvT", [C, DHALL], bf16, kind="ExternalInput")
    wq_d = nc.dram_tensor("wqT", [DIM_S, NHKD], fp32, kind="ExternalInput")
    wp_d = nc.dram_tensor("wpT", [DHALL, DIM_S], fp32, kind="ExternalInput")
    wo_d = nc.dram_tensor("woT", [DIM_S, C], fp32, kind="ExternalInput")
    kb_d = nc.dram_tensor("kb", [NHKD, 1], fp32, kind="ExternalInput")
    qb_d = nc.dram_tensor("qb", [NHKD, 1], fp32, kind="ExternalInput")
    vbb_d = nc.dram_tensor("vbb", [128, DHALL], fp32, kind="ExternalInput")
    ob_d = nc.dram_tensor("ob", [128, 3], fp32, kind="ExternalInput")
    y_d = nc.dram_tensor("y", [C, NH], fp32, kind="ExternalOutput")

    xx_t = xx_d[:, :].rearrange("(t p) n -> t p n", p=128)   # [3,128,N]
    wk_t = wk_d[:, :].rearrange("(t p) m -> t p m", p=128)
    wv_t = wv_d[:, :].rearrange("(t p) m -> t p m", p=128)

    with TileContext(nc) as tc:
        with tc.tile_pool(name="persist", bufs=1) as pp:
            # ---- small weights / constants ----
            wq_sb = pp.tile([DIM_S, NHKD], fp32, tag="wq")
            nc.sync.dma_start(out=wq_sb, in_=wq_d[:, :])
            wp0_sb = pp.tile([128, DIM_S], fp32, tag="wp0")
            nc.sync.dma_start(out=wp0_sb, in_=wp_d[0:128, :])
            wp1_sb = pp.tile([64, DIM_S], fp32, tag="wp1")
            nc.sync.dma_start(out=wp1_sb, in_=wp_d[128:192, :])
            wo_sb = pp.tile([DIM_S, C], fp32, tag="wo")
            nc.sync.dma_start(out=wo_sb, in_=wo_d[:, :])
            kb_sb = pp.tile([NHKD, 1], fp32, tag="kb")
            nc.sync.dma_start(out=kb_sb, in_=kb_d[:, :])
            qb_sb = pp.tile([NHKD, 1], fp32, tag="qb")
            nc.sync.dma_start(out=qb_sb, in_=qb_d[:, :])
            vbb_sb = pp.tile([128, DHALL], fp32, tag="vbb")
            nc.sync.dma_start(out=vbb_sb, in_=vbb_d[:, :])
            ob_sb = pp.tile([128, 3], fp32, tag="ob")
            nc.sync.dma_start(out=ob_sb, in_=ob_d[:, :])
            ones_sb = pp.tile([128, 32], bf16, tag="ones")
            nc.vector.memset(ones_sb, 1.0)
            # preload the exp table set so the first real EXP is cheap
            warm = pp.tile([1, 1], fp32, tag="warm")
            nc.scalar.activation(out=warm, in_=ones_sb[0:1, 0:1], func=AF.Exp)

            wk_sb = [pp.tile([128, NHKD], fp32, tag=f"wk{t}", name=f"wk{t}")
                     for t in range(KT)]
            wv_sb = [pp.tile([128, DHALL], bf16, tag=f"wv{t}", name=f"wv{t}")
                     for t in range(KT)]
            for t in range(KT):
                nc.sync.dma_start(out=wk_sb[t], in_=wk_t[t])
                nc.sync.dma_start(out=wv_sb[t], in_=wv_t[t])

            xh_sb = pp.tile([DIM_S, NH], fp32, tag="xh")
            nc.sync.dma_start(out=xh_sb, in_=xh_d[:, :])

            # persistent attention operands
            # krep rows: {0:khi 16:klo 32:khi | 64: same}  (two strip sets)
            # qrep rows: {0:qhi 16:qhi 32:qlo | 64: same}
            krep = [pp.tile([128, N], bf16, tag=f"krep{h}", name=f"krep{h}")
                    for h in range(NUM_HEADS)]
            qrep = [pp.tile([128, NH], bf16, tag=f"qrep{h}", name=f"qrep{h}")
                    for h in range(NUM_HEADS)]
            vT = [pp.tile([128, DHALL], bf16, tag=f"vt{c}", name=f"vt{c}")
                  for c in range(NMCH)]
            rhs_p0 = pp.tile([128, NH], fp32, tag="rhs0")   # heads 0-3
            rhs_p1 = pp.tile([64, NH], fp32, tag="rhs1")    # heads 4-5

            # ---- conv phase ----
            with tc.tile_pool(name="convin", bufs=1) as cip, \
                 tc.tile_pool(name="convps", bufs=2, space="PSUM") as cps:
                xx_sb = [cip.tile([128, N], fp32, tag=f"xx{t}", name=f"xx{t}")
                         for t in range(KT)]
                for t in range(KT):
                    nc.sync.dma_start(out=xx_sb[t], in_=xx_t[t])
                xx_bf = [cip.tile([128, N], bf16, tag=f"xb{t}", name=f"xb{t}")
                         for t in range(KT)]

                # dense q = WqT.T @ xh + qb, split hi/lo  [96, NH]
                q_hi = cip.tile([NHKD, NH], bf16, tag="q_hi")
                q_lo = cip.tile([NHKD, NH], bf16, tag="q_lo")
                for j in range(NNC):
                    sl = slice(j * NCH, (j + 1) * NCH)
                    psq = cps.tile([NHKD, NCH], fp32, tag="psq")
                    nc.tensor.matmul(out=psq, lhsT=wq_sb, rhs=xh_sb[:, sl],
                                     start=True, stop=True)
                    nc.vector.tensor_scalar(
                        out=q_hi[:, sl], in0=psq, scalar1=qb_sb[:, 0:1],
                        scalar2=None, op0=OP.add)
                    nc.vector.scalar_tensor_tensor(
                        out=q_lo[:, sl], in0=psq, scalar=qb_sb[:, 0:1],
                        in1=q_hi[:, sl], op0=OP.add, op1=OP.subtract)

                # dense k = WkT.T @ xx + kb, split hi/lo  [96, N]
                k_hi = cip.tile([NHKD, N], bf16, tag="k_hi")
                k_lo = cip.tile([NHKD, N], bf16, tag="k_lo")
                for j in range(N // NCH):
                    sl = slice(j * NCH, (j + 1) * NCH)
                    psk = cps.tile([NHKD, NCH], fp32, tag="psk")
                    for t in range(KT):
                        nc.tensor.matmul(
                            out=psk, lhsT=wk_sb[t], rhs=xx_sb[t][:, sl],
                            start=(t == 0), stop=(t == KT - 1))
                    nc.vector.tensor_scalar(
                        out=k_hi[:, sl], in0=psk, scalar1=kb_sb[:, 0:1],
                        scalar2=None, op0=OP.add)
                    nc.vector.scalar_tensor_tensor(
                        out=k_lo[:, sl], in0=psk, scalar=kb_sb[:, 0:1],
                        in1=k_hi[:, sl], op0=OP.add, op1=OP.subtract)

                # replicate into the two 48-row strip sets
                for h in range(NUM_HEADS):
                    hs = slice(KD * h, KD * (h + 1))
                    for bp in (0, 64):
                        nc.sync.dma_start(out=qrep[h][bp:bp + 16, :],
                                          in_=q_hi[hs, :])
                        nc.sync.dma_start(out=qrep[h][bp + 16:bp + 32, :],
                                          in_=q_hi[hs, :])
                        nc.sync.dma_start(out=qrep[h][bp + 32:bp + 48, :],
                                          in_=q_lo[hs, :])
                        nc.sync.dma_start(out=krep[h][bp:bp + 16, :],
                                          in_=k_hi[hs, :])
                        nc.sync.dma_start(out=krep[h][bp + 16:bp + 32, :],
                                          in_=k_lo[hs, :])
                        nc.sync.dma_start(out=krep[h][bp + 32:bp + 48, :],
                                          in_=k_hi[hs, :])

                # bf16 xx for the vT conv (off the critical path)
                for t in range(KT):
                    nc.vector.tensor_copy(out=xx_bf[t], in_=xx_sb[t])

                # vT chunks: vT[m,d] = xx_chunk^T @ WvT (+ vb broadcast)
                for c in range(NMCH):
                    psv = cps.tile([128, DHALL], fp32, tag="psv")
                    for t in range(KT):
                        nc.tensor.matmul(
                            out=psv,
                            lhsT=xx_bf[t][:, c * MCH:(c + 1) * MCH],
                            rhs=wv_sb[t],
                            start=(t == 0), stop=(t == KT - 1))
                    nc.vector.tensor_tensor(
                        out=vT[c], in0=psv, in1=vbb_sb, op=OP.add)

            # ---- attention + fused output stage ----
            with tc.tile_pool(name="attn_sb", bufs=1) as asb, \
                 tc.tile_pool(name="attn_ps", bufs=1, space="PSUM") as aps:
                for j in range(NNC):           # query chunk
                    nsl = slice(j * NCH, (j + 1) * NCH)
                    e_t = {}
                    for h in range(NUM_HEADS):
                        e_t[h] = asb.tile([128, NMCH * NCH], bf16,
                                          tag="e", bufs=7, name=f"e{h}")
                        for g in range(NG):    # 3 key-chunks per group
                            pss = aps.tile([128, 3 * 512], fp32,
                                           tag="pss", bufs=2)
                            for s in range(MG):
                                c = MG * g + s
                                bp = 64 * (c % 2)
                                nc.tensor.matmul(
                                    out=pss[:, 512 * s:512 * s + NCH],
                                    lhsT=krep[h][bp:bp + 48,
                                                 c * MCH:(c + 1) * MCH],
                                    rhs=qrep[h][bp:bp + 48, nsl],
                                    start=True, stop=True)
                            src = pss.rearrange("p (b n) -> p b n", n=512)
                            dst = e_t[h][:, g * MG * NCH:(g + 1) * MG * NCH]
                            nc.scalar.activation(
                                out=dst.rearrange("p (b n) -> p b n", n=NCH),
                                in_=src[:, :, 0:NCH], func=AF.Exp)

                    # AV + denominator, col-tiled
                    for pi, heads in enumerate(((0, 1, 2, 3), (4, 5))):
                        npart = 32 * len(heads)
                        av = aps.tile([128, NCH], fp32, tag="av")
                        dn = aps.tile([128, NCH], fp32, tag="dn")
                        for c in range(NMCH):
                            st, sp = (c == 0), (c == NMCH - 1)
                            for i, h in enumerate(heads):
                                nc.tensor.matmul(
                                    out=av[32 * i:32 * i + 32, :],
                                    lhsT=vT[c][:, 32 * h:32 * h + 32],
                                    rhs=e_t[h][:, c * NCH:(c + 1) * NCH],
                                    start=st, stop=sp, skip_group_check=True,
                                    tile_position=(0, 32 * i))
                            for i, h in enumerate(heads):
                                nc.tensor.matmul(
                                    out=dn[32 * i:32 * i + 32, :],
                                    lhsT=ones_sb[:, 0:32],
                                    rhs=e_t[h][:, c * NCH:(c + 1) * NCH],
                                    start=st, stop=sp, skip_group_check=True,
                                    tile_position=(0, 32 * i))
                        recip = asb.tile([128, NCH], fp32, tag="recip",
                                         bufs=2)
                        nc.vector.reciprocal_approx_fast(
                            out=recip[:npart], in_=dn[:npart])
                        dst = rhs_p0 if pi == 0 else rhs_p1
                        nc.vector.scalar_tensor_tensor(
                            out=dst[:npart, nsl], in0=av[:npart], scalar=0.0,
                            in1=recip[:npart], op0=OP.max, op1=OP.mult)

                    # fused Wp -> +x -> Wo -> relu -> DMA for this chunk
                    psp = aps.tile([DIM_S, NCH], fp32, tag="dn")
                    nc.tensor.matmul(out=psp, lhsT=wp0_sb,
                                     rhs=rhs_p0[:, nsl], start=True,
                                     stop=False)
                    nc.tensor.matmul(out=psp, lhsT=wp1_sb,
                                     rhs=rhs_p1[:, nsl], start=False,
                                     stop=True)
                    xres = asb.tile([DIM_S, NCH], fp32, tag="xres", bufs=2)
                    nc.vector.tensor_tensor(
                        out=xres, in0=psp, in1=xh_sb[:, nsl], op=OP.add)
                    for g in range(3):
                        psy = aps.tile([128, NCH], fp32, tag="av")
                        nc.tensor.matmul(
                            out=psy, lhsT=wo_sb[:, 128 * g:128 * (g + 1)],
                            rhs=xres, start=True, stop=True)
                        ysb = asb.tile([128, NCH], fp32, tag="ysb", bufs=3)
                        nc.vector.tensor_scalar(
                            out=ysb, in0=psy, scalar1=ob_sb[:, g:g + 1],
                            scalar2=0.0, op0=OP.add, op1=OP.max)
                        nc.sync.dma_start(
                            out=y_d[128 * g:128 * (g + 1), nsl], in_=ysb)
    return nc


def kernel(**inputs):
    import os
    from concourse.bass_utils import run_bass_kernel_spmd
    from ml_dtypes import bfloat16

    xx = np.asarray(inputs["xx"], dtype=np.float32)
    Wq = np.asarray(inputs["Wq"], dtype=np.float32)
    Wk = np.asarray(inputs["Wk"], dtype=np.float32)
    Wv = np.asarray(inputs["Wv"], dtype=np.float32)
    Wp = np.asarray(inputs["Wp"], dtype=np.float32)
    Wo = np.asarray(inputs["Wo"], dtype=np.float32)

    wqT = np.ascontiguousarray((inputs["q_scale"][:, None] * Wq).T)
    wkT = np.ascontiguousarray((inputs["k_scale"][:, None] * Wk).T)
    wvT = np.ascontiguousarray(
        (inputs["v_scale"][:, None] * Wv).T).astype(bfloat16)
    wpT = np.ascontiguousarray((inputs["p_scale"][:, None] * Wp).T)
    Wo2 = inputs["o_scale"][:, None] * Wo
    woT = np.ascontiguousarray(Wo2.T)
    ob2 = inputs["o_bias"] + Wo2 @ inputs["p_bias"]
    ob = np.ascontiguousarray(ob2.reshape(3, 128).T)   # [128, 3]
    vbb = np.ascontiguousarray(np.tile(inputs["v_bias"][None, :], (128, 1)))
    kb = np.ascontiguousarray(inputs["k_bias"][:, None])
    qb = np.ascontiguousarray(inputs["q_bias"][:, None])

    xx_flat = xx.reshape(B, C, N)
    shared = dict(wkT=wkT.astype(np.float32), wvT=wvT,
                  wqT=wqT.astype(np.float32),
                  wpT=wpT.astype(np.float32), woT=woT.astype(np.float32),
                  kb=kb.astype(np.float32), qb=qb.astype(np.float32),
                  vbb=vbb.astype(np.float32), ob=ob.astype(np.float32))

    in_maps = []
    for core in range(NCORES):
        b, half = core // 2, core % 2
        xxb = np.ascontiguousarray(xx_flat[b])
        xh = np.ascontiguousarray(
            xx_flat[b][3 * DIM_S:, half * NH:(half + 1) * NH])
        in_maps.append(dict(xx=xxb, xh=xh, **shared))

    nc = build_nc()
    if not nc.is_finalized():
        nc.finalize()
    trace = bool(int(os.environ.get("KERNEL_TRACE", "0")))
    res = run_bass_kernel_spmd(nc, in_maps, list(range(NCORES)),
                               trace=trace)
    if trace:
        kernel.last_result = res

    out = np.empty((B, C, N), dtype=np.float32)
    for core in range(NCORES):
        b, half = core // 2, core % 2
        out[b][:, half * NH:(half + 1) * NH] = res.results[core]["y"]
    return out.reshape(B, C, Himg, Wimg)


# revision 22
# speedup vs baseline: 1.0683x; 1.0683x over previous
"""Trainium2 Bass kernel for CrossTrans block (dense_transformer).

Computation (per batch b):
  x   = xx[:, 288:384]                      # query stream  [96, N]
  q   = Wq'@x + qb ; k = Wk'@xx + kb ; v = Wv'@xx + vb
  attn= softmax(q_h^T k_h) per head ; av = v_h @ attn^T
  y   = relu(Wo'@(x + Wp'@relu(av_norm)) + ob')
BN scales folded into weights on host; p_bias folded into o_bias.

Sharding: 8 cores = 4 batches x 2 query-halves; k/v recomputed per half.

Device layout: scores transposed [keys->partitions, queries->free] so AV
consumes exp(scores) without transposing the attention matrix. Softmax
denominators via ones-matmul, col-tiled, row-replicated x32 so they align
partition-wise with AV output for the normalize op. exp skips max
subtraction (|logit| <~ 70 fits fp32/bf16 range).

Precision: scores use a 3-term split-bf16 matmul
  [k_hi; k_lo; k_hi]^T . [q_hi; q_hi; q_lo]  (K=48, fp32-grade logits)
with the terms stacked at two 64-row strips for tile_position concurrency.
e and v are bf16; k/q convs and the output stage are fp32.
"""

import numpy as np

NUM_HEADS = 6
KD = 16
DH = 32
B, C, Himg, Wimg = 4, 384, 48, 48
N = Himg * Wimg          # 2304
NH = N // 2              # 1152 queries per core
DIM_S = C // 4           # 96
NHKD = NUM_HEADS * KD    # 96
DHALL = NUM_HEADS * DH   # 192
NCORES = 8

NCH = 384                # query chunk (free dim of score matmuls)
NNC = NH // NCH          # 3 query chunks per core
MCH = 128                # key chunk (partition tile)
NMCH = N // MCH          # 18 key chunks
MG = 3                   # key chunks per exp group (3 psum banks)
NG = NMCH // MG          # 6 groups
KT = C // 128            # 3 contraction tiles over channels


def build_nc():
    import concourse.bacc as bacc
    import concourse.mybir as mybir
    from concourse.tile import TileContext

    fp32 = mybir.dt.float32
    bf16 = mybir.dt.bfloat16
    AF = mybir.ActivationFunctionType
    OP = mybir.AluOpType

    nc = bacc.Bacc("TRN2", target_bir_lowering=False)

    xx_d = nc.dram_tensor("xx", [C, N], fp32, kind="ExternalInput")
    xh_d = nc.dram_tensor("xh", [DIM_S, NH], fp32, kind="ExternalInput")
    wk_d = nc.dram_tensor("wkT", [C, NHKD], fp32, kind="ExternalInput")
    wv_d = nc.dram_tensor("wvT", [C, DHALL], bf16, kind="ExternalInput")
    wq_d = nc.dram_tensor("wqT", [DIM_S, NHKD], fp32, kind="ExternalInput")
    wp_d = nc.dram_tensor("wpT", [DHALL, DIM_S], fp32, kind="ExternalInput")
    wo_d = nc.dram_tensor("woT", [DIM_S, C], fp32, kind="ExternalInput")
    kb_d = nc.dram_tensor("kb", [NHKD, 1], fp32, kind="ExternalInput")
    qb_d = nc.dram_tensor("qb", [NHKD, 1], fp32, kind="ExternalInput")
    vbb_d = nc.dram_tensor("vbb", [128, DHALL], fp32, kind="ExternalInput")
    ob_d = nc.dram_tensor("ob", [128, 3], fp32, kind="ExternalInput")
    y_d = nc.dram_tensor("y", [C, NH], fp32, kind="ExternalOutput")

    xx_t = xx_d[:, :].rearrange("(t p) n -> t p n", p=128)   # [3,128,N]
    wk_t = wk_d[:, :].rearrange("(t p) m -> t p m", p=128)
    wv_t = wv_d[:, :].rearrange("(t p) m -> t p m", p=128)

    with TileContext(nc) as tc:
        with tc.tile_pool(name="persist", bufs=1) as pp:
            # ---- small weights / constants ----
            wq_sb = pp.tile([DIM_S, NHKD], fp32, tag="wq")
            nc.sync.dma_start(out=wq_sb, in_=wq_d[:, :])
            wp0_sb = pp.tile([128, DIM_S], fp32, tag="wp0")
            nc.sync.dma_start(out=wp0_sb, in_=wp_d[0:128, :])
            wp1_sb = pp.tile([64, DIM_S], fp32, tag="wp1")
            nc.sync.dma_start(out=wp1_sb, in_=wp_d[128:192, :])
            wo_sb = pp.tile([DIM_S, C], fp32, tag="wo")
            nc.sync.dma_start(out=wo_sb, in_=wo_d[:, :])
            kb_sb = pp.tile([NHKD, 1], fp32, tag="kb")
            nc.sync.dma_start(out=kb_sb, in_=kb_d[:, :])
            qb_sb = pp.tile([NHKD, 1], fp32, tag="qb")
            nc.sync.dma_start(out=qb_sb, in_=qb_d[:, :])
            vbb_sb = pp.tile([128, DHALL], fp32, tag="vbb")
            nc.sync.dma_start(out=vbb_sb, in_=vbb_d[:, :])
            ob_sb = pp.tile([128, 3], fp32, tag="ob")
            nc.sync.dma_start(out=ob_sb, in_=ob_d[:, :])
            ones_sb = pp.tile([128, 32], bf16, tag="ones")
            nc.vector.memset(ones_sb, 1.0)
            # preload the exp table set so the first real EXP is cheap
            warm = pp.tile([1, 1], fp32, tag="warm")
            nc.scalar.activation(out=warm, in_=ones_sb[0:1, 0:1], func=AF.Exp)

            wk_sb = [pp.tile([128, NHKD], fp32, tag=f"wk{t}", name=f"wk{t}")
                     for t in range(KT)]
            wv_sb = [pp.tile([128, DHALL], bf16, tag=f"wv{t}", name=f"wv{t}")
                     for t in range(KT)]
            for t in range(KT):
                nc.sync.dma_start(out=wk_sb[t], in_=wk_t[t])
                nc.sync.dma_start(out=wv_sb[t], in_=wv_t[t])

            xh_sb = pp.tile([DIM_S, NH], fp32, tag="xh")
            nc.sync.dma_start(out=xh_sb, in_=xh_d[:, :])

            # persistent attention operands
            # krep rows: {0:khi 16:klo 32:khi | 64: same}  (two strip sets)
            # qrep rows: {0:qhi 16:qhi 32:qlo | 64: same}
            krep = [pp.tile([128, N], bf16, tag=f"krep{h}", name=f"krep{h}")
                    for h in range(NUM_HEADS)]
            qrep = [pp.tile([128, NH], bf16, tag=f"qrep{h}", name=f"qrep{h}")
                    for h in range(NUM_HEADS)]
            vT = [pp.tile([128, DHALL], bf16, tag=f"vt{c}", name=f"vt{c}")
                  for c in range(NMCH)]
            rhs_p0 = pp.tile([128, NH], fp32, tag="rhs0")   # heads 0-3
            rhs_p1 = pp.tile([64, NH], fp32, tag="rhs1")    # heads 4-5

            # ---- conv phase ----
            with tc.tile_pool(name="convxx", bufs=1) as cxp, \
                 tc.tile_pool(name="convst", bufs=1) as cip, \
                 tc.tile_pool(name="convps", bufs=2, space="PSUM") as cps:
                xx_sb = [cxp.tile([128, N], fp32, tag=f"xx{t}", name=f"xx{t}")
                         for t in range(KT)]
                for t in range(KT):
                    half = N // 2
                    nc.sync.dma_start(out=xx_sb[t][:, 0:half],
                                      in_=xx_t[t][:, 0:half])
                    nc.sync.dma_start(out=xx_sb[t][:, half:N],
                                      in_=xx_t[t][:, half:N])
                xx_bf = [cip.tile([128, N], bf16, tag=f"xb{t}", name=f"xb{t}")
                         for t in range(KT)]

                # dense q = WqT.T @ xh + qb, split hi/lo  [96, NH]
                q_hi = cip.tile([NHKD, NH], bf16, tag="q_hi")
                q_lo = cip.tile([NHKD, NH], bf16, tag="q_lo")
                for j in range(NNC):
                    sl = slice(j * NCH, (j + 1) * NCH)
                    psq = cps.tile([NHKD, NCH], fp32, tag="psq")
                    nc.tensor.matmul(out=psq, lhsT=wq_sb, rhs=xh_sb[:, sl],
                                     start=True, stop=True)
                    nc.vector.tensor_scalar(
                        out=q_hi[:, sl], in0=psq, scalar1=qb_sb[:, 0:1],
                        scalar2=None, op0=OP.add)
                    nc.vector.scalar_tensor_tensor(
                        out=q_lo[:, sl], in0=psq, scalar=qb_sb[:, 0:1],
                        in1=q_hi[:, sl], op0=OP.add, op1=OP.subtract)

                # dense k = WkT.T @ xx + kb, split hi/lo  [96, N]
                k_hi = cip.tile([NHKD, N], bf16, tag="k_hi")
                k_lo = cip.tile([NHKD, N], bf16, tag="k_lo")
                for j in range(N // NCH):
                    sl = slice(j * NCH, (j + 1) * NCH)
                    psk = cps.tile([NHKD, NCH], fp32, tag="psk")
                    for t in range(KT):
                        nc.tensor.matmul(
                            out=psk, lhsT=wk_sb[t], rhs=xx_sb[t][:, sl],
                            start=(t == 0), stop=(t == KT - 1))
                    nc.vector.tensor_scalar(
                        out=k_hi[:, sl], in0=psk, scalar1=kb_sb[:, 0:1],
                        scalar2=None, op0=OP.add)
                    nc.vector.scalar_tensor_tensor(
                        out=k_lo[:, sl], in0=psk, scalar=kb_sb[:, 0:1],
                        in1=k_hi[:, sl], op0=OP.add, op1=OP.subtract)

                # replicate into the two 48-row strip sets,
                # spread across several engines' DMA queues
                dmae = [nc.sync, nc.scalar]
                di = 0
                for h in range(NUM_HEADS):
                    hs = slice(KD * h, KD * (h + 1))
                    for bp in (0, 64):
                        for dst_t, s0, src_t in (
                                (qrep[h], bp, q_hi), (qrep[h], bp + 16, q_hi),
                                (qrep[h], bp + 32, q_lo),
                                (krep[h], bp, k_hi), (krep[h], bp + 16, k_lo),
                                (krep[h], bp + 32, k_hi)):
                            dmae[di % 2].dma_start(
                                out=dst_t[s0:s0 + 16, :], in_=src_t[hs, :])
                            di += 1

                # bf16 xx for the vT conv (off the critical path)
                for t in range(KT):
                    nc.vector.tensor_copy(out=xx_bf[t], in_=xx_sb[t])

                # vT chunks: vT[m,d] = xx_chunk^T @ WvT (+ vb broadcast)
                for c in range(NMCH):
                    psv = cps.tile([128, DHALL], fp32, tag="psv")
                    for t in range(KT):
                        nc.tensor.matmul(
                            out=psv,
                            lhsT=xx_bf[t][:, c * MCH:(c + 1) * MCH],
                            rhs=wv_sb[t],
                            start=(t == 0), stop=(t == KT - 1))
                    nc.vector.tensor_tensor(
                        out=vT[c], in0=psv, in1=vbb_sb, op=OP.add)

            # ---- attention + fused output stage ----
            with tc.tile_pool(name="attn_sb", bufs=1) as asb, \
                 tc.tile_pool(name="attn_ps", bufs=1, space="PSUM") as aps:
                for j in range(NNC):           # query chunk
                    nsl = slice(j * NCH, (j + 1) * NCH)
                    e_t = {}
                    for h in range(NUM_HEADS):
                        e_t[h] = asb.tile([128, NMCH * NCH], bf16,
                                          tag="e", bufs=5, name=f"e{h}")
                        for g in range(NG):    # 3 key-chunks per group
                            pss = aps.tile([128, 3 * 512], fp32,
                                           tag="pss", bufs=2)
                            for s in range(MG):
                                c = MG * g + s
                                bp = 64 * (c % 2)
                                nc.tensor.matmul(
                                    out=pss[:, 512 * s:512 * s + NCH],
                                    lhsT=krep[h][bp:bp + 48,
                                                 c * MCH:(c + 1) * MCH],
                                    rhs=qrep[h][bp:bp + 48, nsl],
                                    start=True, stop=True)
                            src = pss.rearrange("p (b n) -> p b n", n=512)
                            dst = e_t[h][:, g * MG * NCH:(g + 1) * MG * NCH]
                            nc.scalar.activation(
                                out=dst.rearrange("p (b n) -> p b n", n=NCH),
                                in_=src[:, :, 0:NCH], func=AF.Exp)

                    # AV + denominator, col-tiled
                    for pi, heads in enumerate(((0, 1, 2, 3), (4, 5))):
                        npart = 32 * len(heads)
                        av = aps.tile([128, NCH], fp32, tag="av")
                        dn = aps.tile([128, NCH], fp32, tag="dn")
                        for c in range(NMCH):
                            st, sp = (c == 0), (c == NMCH - 1)
                            for i, h in enumerate(heads):
                                nc.tensor.matmul(
                                    out=av[32 * i:32 * i + 32, :],
                                    lhsT=vT[c][:, 32 * h:32 * h + 32],
                                    rhs=e_t[h][:, c * NCH:(c + 1) * NCH],
                                    start=st, stop=sp, skip_group_check=True,
                                    tile_position=(0, 32 * i))
                            for i, h in enumerate(heads):
                                nc.tensor.matmul(
                                    out=dn[32 * i:32 * i + 32, :],
                                    lhsT=ones_sb[:, 0:32],
                                    rhs=e_t[h][:, c * NCH:(c + 1) * NCH],
                                    start=st, stop=sp, skip_group_check=True,
                                    tile_position=(0, 32 * i))
                        recip = asb.tile([128, NCH], fp32, tag="recip",
                                         bufs=2)
                        nc.vector.reciprocal_approx_fast(
                            out=recip[:npart], in_=dn[:npart])
                        dst = rhs_p0 if pi == 0 else rhs_p1
                        nc.vector.scalar_tensor_tensor(
                            out=dst[:npart, nsl], in0=av[:npart], scalar=0.0,
                            in1=recip[:npart], op0=OP.max, op1=OP.mult)

                    # fused Wp -> +x -> Wo -> relu -> DMA for this chunk
                    psp = aps.tile([DIM_S, NCH], fp32, tag="dn")
                    nc.tensor.matmul(out=psp, lhsT=wp0_sb,
                                     rhs=rhs_p0[:, nsl], start=True,
                                     stop=False)
                    nc.tensor.matmul(out=psp, lhsT=wp1_sb,
                                     rhs=rhs_p1[:, nsl], start=False,
                                     stop=True)
                    xres = asb.tile([DIM_S, NCH], fp32, tag="xres", bufs=2)
                    nc.vector.tensor_tensor(
                        out=xres, in0=psp, in1=xh_sb[:, nsl], op=OP.add)
                    for g in range(3):
                        psy = aps.tile([128, NCH], fp32, tag="av")
                        nc.tensor.matmul(
                            out=psy, lhsT=wo_sb[:, 128 * g:128 * (g + 1)],
                            rhs=xres, start=True, stop=True)
                        ysb = asb.tile([128, NCH], fp32, tag="ysb", bufs=3)
                        nc.vector.tensor_scalar(
                            out=ysb, in0=psy, scalar1=ob_sb[:, g:g + 1],
                            scalar2=0.0, op0=OP.add, op1=OP.max)
                        nc.sync.dma_start(
                            out=y_d[128 * g:128 * (g + 1), nsl], in_=ysb)
    return nc


def kernel(**inputs):
    import os
    from concourse.bass_utils import run_bass_kernel_spmd
    from ml_dtypes import bfloat16

    xx = np.asarray(inputs["xx"], dtype=np.float32)
    Wq = np.asarray(inputs["Wq"], dtype=np.float32)
    Wk = np.asarray(inputs["Wk"], dtype=np.float32)
    Wv = np.asarray(inputs["Wv"], dtype=np.float32)
    Wp = np.asarray(inputs["Wp"], dtype=np.float32)
    Wo = np.asarray(inputs["Wo"], dtype=np.float32)

    wqT = np.ascontiguousarray((inputs["q_scale"][:, None] * Wq).T)
    wkT = np.ascontiguousarray((inputs["k_scale"][:, None] * Wk).T)
    wvT = np.ascontiguousarray(
        (inputs["v_scale"][:, None] * Wv).T).astype(bfloat16)
    wpT = np.ascontiguousarray((inputs["p_scale"][:, None] * Wp).T)
    Wo2 = inputs["o_scale"][:, None] * Wo
    woT = np.ascontiguousarray(Wo2.T)
    ob2 = inputs["o_bias"] + Wo2 @ inputs["p_bias"]
    ob = np.ascontiguousarray(ob2.reshape(3, 128).T)   # [128, 3]
    vbb = np.ascontiguousarray(np.tile(inputs["v_bias"][None, :], (128, 1)))
    kb = np.ascontiguousarray(inputs["k_bias"][:, None])
    qb = np.ascontiguousarray(inputs["q_bias"][:, None])

    xx_flat = xx.reshape(B, C, N)
    shared = dict(wkT=wkT.astype(np.float32), wvT=wvT,
                  wqT=wqT.astype(np.float32),
                  wpT=wpT.astype(np.float32), woT=woT.astype(np.float32),
                  kb=kb.astype(np.float32), qb=qb.astype(np.float32),
                  vbb=vbb.astype(np.float32), ob=ob.astype(np.float32))

    in_maps = []
    for core in range(NCORES):
        b, half = core // 2, core % 2
        xxb = np.ascontiguousarray(xx_flat[b])
        xh = np.ascontiguousarray(
            xx_flat[b][3 * DIM_S:, half * NH:(half + 1) * NH])
        in_maps.append(dict(xx=xxb, xh=xh, **shared))

    nc = build_nc()
    if not nc.is_finalized():
        nc.finalize()
    trace = bool(int(os.environ.get("KERNEL_TRACE", "0")))
    res = run_bass_kernel_spmd(nc, in_maps, list(range(NCORES)),
                               trace=trace)
    if trace:
        kernel.last_result = res

    out = np.empty((B, C, N), dtype=np.float32)
    for core in range(NCORES):
        b, half = core // 2, core % 2
        out[b][:, half * NH:(half + 1) * NH] = res.results[core]["y"]
    return out.reshape(B, C, Himg, Wimg)
